# revision 22
# baseline (speedup 1.0000x reference)
"""Trainium2 Bass kernel for nn_Batch_Edge (gnn_message_passing).

Computation (see reference):
    node_embed = last_node_batch @ W_embed + b_embed          # [B, H]
    stack      = concat([h, node_embed[seg]], axis=1)         # [N, 2H]
    out        = tanh(stack @ W1 + b1); out = tanh(out @ W2 + b2)
    edges      = out @ W3 + b3                                # [N, 2]
    result     = edges reshaped to [B, max_nodes*2]  (no padding: all graphs full)

Strategy: shard 512 graphs (131072 nodes) contiguously across 8 cores (64
graphs / 16384 nodes each). All activations are kept feature-on-partition
([feature, node]) so matmuls need no on-device transposes; the host supplies
h and last_node_batch pre-transposed. The per-graph embedding contribution
C = node_embed @ W1[H:, :] + b1 is computed once per core ([256, 64]) and
added to the L1 PSUM by DVE as a per-partition broadcast. Matmuls run in
fp32r (fp32 with mantissa rounded to 11 bits — full PE rate); inputs are
pre-rounded on the host so no device cycles are spent rounding.
"""

import os
import numpy as np

B = 512
NPG = 256               # nodes per graph
N = B * NPG             # 131072
HID = 128
NCORES = 8
GPC = B // NCORES       # 64 graphs per core
NPC = N // NCORES       # 16384 nodes per core
PAD_VALUE = -10000.0

ST = 2048               # supertile: nodes handled per main-loop iteration
NST = NPC // ST         # 8 supertiles per core

LAST_RESULT = None      # BassKernelResults of the most recent device run
_CACHE = {}


def _round_fp32r(x: np.ndarray) -> np.ndarray:
    """Round fp32 mantissa to 11 explicit bits (round-half-to-even) — the
    fp32r encoding the PE consumes at full rate."""
    u = np.ascontiguousarray(x, dtype=np.float32).view(np.uint32)
    lsb = (u >> np.uint32(12)) & np.uint32(1)
    r = (u + np.uint32(0x7FF) + lsb) & np.uint32(0xFFFFF000)
    return r.view(np.float32)


def _numpy_ref(last_node_batch, h, W_embed, b_embed, W1, b1, W2, b2, W3, b3,
               segment_ids, max_nodes):
    """Exact host fallback (used only if inputs don't match the expected
    uniform-graph structure)."""
    lnb = np.asarray(last_node_batch, np.float32)
    h = np.asarray(h, np.float32)
    seg = np.asarray(segment_ids).astype(np.int64)
    b = lnb.shape[0]
    n = h.shape[0]
    mn = int(np.asarray(max_nodes))
    node_embed = lnb @ np.asarray(W_embed, np.float32) + np.asarray(b_embed, np.float32)
    stack = np.concatenate([h, node_embed[seg]], axis=1)
    out = np.tanh(stack @ np.asarray(W1, np.float32) + np.asarray(b1, np.float32))
    out = np.tanh(out @ np.asarray(W2, np.float32) + np.asarray(b2, np.float32))
    edges = out @ np.asarray(W3, np.float32) + np.asarray(b3, np.float32)
    counts = np.zeros(b, np.int64)
    np.add.at(counts, seg, 1)
    offsets = np.cumsum(counts) - counts
    pos = np.arange(n) - offsets[seg]
    padded = np.full((b, mn, 2), PAD_VALUE, np.float32)
    padded[seg, pos] = edges
    return padded.reshape(b, mn * 2)


def _build():
    """Build + compile the per-core Bass program (identical on all cores)."""
    import concourse.bacc as bacc
    import concourse.mybir as mybir
    import concourse.tile as tile

    f32 = mybir.dt.float32
    f32r = mybir.dt.float32r
    Tanh = mybir.ActivationFunctionType.Tanh

    nc = bacc.Bacc("TRN2", target_bir_lowering=False, debug=False)

    # wpk layout (free dim): we[0:128] w1t[128:384] w1b[384:640] w2a[640:896]
    #                        w2b[896:1152] w3a[1152:1154] w3b[1154:1156]
    #                        lnbT[1156:1220]
    hT = nc.dram_tensor("hT", [128, NPC], f32r, kind="ExternalInput").ap()
    wpk = nc.dram_tensor("wpk", [128, 1220], f32r, kind="ExternalInput").ap()
    # bpk columns: be, b1a, b1b, b2a, b2b
    bpk = nc.dram_tensor("bpk", [128, 5], f32, kind="ExternalInput").ap()
    out_d = nc.dram_tensor("out", [2, NPC], f32, kind="ExternalOutput").ap()

    with tile.TileContext(nc) as tc:
        with (
            tc.tile_pool(name="w", bufs=1) as wp,
            tc.tile_pool(name="io", bufs=2) as io,
            tc.tile_pool(name="act", bufs=2) as ac,
            tc.tile_pool(name="ps1", bufs=2, space="PSUM") as ps1,
            tc.tile_pool(name="ps2", bufs=1, space="PSUM") as ps2,
            tc.tile_pool(name="ps3", bufs=1, space="PSUM") as ps3,
        ):
            # weights first (small, unblock the prologue), then h chunk 0
            s_w = wp.tile([128, 1220], f32r, tag="wpk")
            nc.sync.dma_start(out=s_w[:], in_=wpk)
            s_b = wp.tile([128, 5], f32, tag="bpk")
            nc.sync.dma_start(out=s_b[:], in_=bpk)
            h_tiles = {}
            t_h0 = io.tile([128, ST], f32r, tag="h")
            nc.sync.dma_start(out=t_h0[:, 0:ST // 2], in_=hT[:, 0:ST // 2])
            nc.sync.dma_start(out=t_h0[:, ST // 2:ST], in_=hT[:, ST // 2:ST])
            h_tiles[0] = t_h0
            s_we = s_w[:, 0:128]
            s_w1t = s_w[:, 128:384]
            s_w1b = s_w[:, 384:640]
            s_w2a = s_w[:, 640:896]
            s_w2b = s_w[:, 896:1152]
            s_w3a = s_w[:, 1152:1154]
            s_w3b = s_w[:, 1154:1156]
            s_lnb = s_w[:, 1156:1220]
            s_be = s_b[:, 0:1]
            s_b1 = [s_b[:, 1:2], s_b[:, 2:3]]
            s_b2 = [s_b[:, 3:4], s_b[:, 4:5]]

            # node_embed^T = W_embed.T @ lnb^T + b_embed   [128, GPC]
            p_e = ps1.tile([128, GPC], f32, tag="ps1")
            nc.tensor.matmul(p_e[:], s_we, s_lnb, start=True, stop=True)
            s_emb = wp.tile([128, GPC], f32r, tag="emb")
            nc.vector.tensor_scalar_add(s_emb[:], p_e[:], s_be)

            # C^T halves = (W1[H:, :].T @ node_embed^T + b1)  each [128, GPC]
            s_ct = []
            for m in (0, 1):
                p_c = ps1.tile([128, GPC], f32, tag="ps1")
                nc.tensor.matmul(
                    p_c[:], s_w1b[:, 128 * m:128 * m + 128], s_emb[:],
                    start=True, stop=True,
                )
                t = wp.tile([128, GPC], f32, tag=f"ct{m}")
                nc.vector.tensor_scalar_add(t[:], p_c[:], s_b1[m])
                s_ct.append(t)

            for st in range(NST):
                if st in h_tiles:
                    t_h = h_tiles.pop(st)
                else:
                    t_h = io.tile([128, ST], f32r, tag="h")
                    nc.sync.dma_start(
                        out=t_h[:], in_=hT[:, st * ST:(st + 1) * ST],
                    )

                # L1: y1[m] = tanh(W1[:H, m].T @ h^T + C[m][:, g]);
                # both halves staged into one tile so tanh is a single
                # [128, 4096] ACT instruction.
                y1s = ac.tile([128, 2 * ST], f32r, tag="y1s")
                for m in (0, 1):
                    for j in range(ST // 512):
                        p1 = ps1.tile([128, 512], f32, tag="ps1")
                        nc.tensor.matmul(
                            p1[:], s_w1t[:, 128 * m:128 * m + 128],
                            t_h[:, 512 * j:512 * j + 512],
                            start=True, stop=True,
                        )
                        g = st * (ST // NPG) + j * 2
                        o = m * ST + 512 * j
                        nc.vector.tensor_tensor(
                            y1s[:, o:o + 512]
                            .rearrange("p (a b) -> p a b", a=2),
                            p1[:].rearrange("p (a b) -> p a b", a=2),
                            s_ct[m][:, g:g + 2].broadcast_to((128, 2, 256)),
                            mybir.AluOpType.add,
                        )
                y1t = ac.tile([128, 2 * ST], f32r, tag="y1")
                for ci in (0, 1):
                    nc.scalar.activation(
                        y1t[:].rearrange("p (a b) -> p a b", a=4)[:, ci::2, :],
                        y1s[:].rearrange("p (a b) -> p a b", a=4)[:, ci::2, :],
                        Tanh,
                    )
                y1 = [y1t[:, 0:ST], y1t[:, ST:2 * ST]]

                # L2: y2[m] = tanh(W2[:, m].T @ y1 + b2[m])
                y2 = []
                for m in (0, 1):
                    yt = ac.tile([128, ST], f32r, tag=f"y2{m}")
                    p2 = ps2.tile([128, ST], f32, tag="ps2")
                    for j2 in range(ST // 512):
                        po = 512 * j2
                        nc.tensor.matmul(
                            p2[:, po:po + 512],
                            s_w2a[:, 128 * m:128 * m + 128],
                            y1[0][:, po:po + 512],
                            start=True, stop=False,
                        )
                        nc.tensor.matmul(
                            p2[:, po:po + 512],
                            s_w2b[:, 128 * m:128 * m + 128],
                            y1[1][:, po:po + 512],
                            start=False, stop=True,
                        )
                    for ci in (0, 1):
                        nc.scalar.activation(
                            yt[:].rearrange("p (a b) -> p a b", a=2)[:, ci, :],
                            p2[:].rearrange("p (a b) -> p a b", a=2)[:, ci, :],
                            Tanh, bias=s_b2[m],
                        )
                    y2.append(yt)

                # L3: edges^T[:, n] = W3.T @ y2[:, n]; two 512-col pairs per
                # [2, 1024] PSUM tile, one DVE copy each.
                ed_t = io.tile([2, ST], f32, tag="out")
                for jj in range(ST // 1024):
                    p3 = ps3.tile([2, 1024], f32, tag="ps3")
                    for j2 in (0, 1):
                        sl = 1024 * jj + 512 * j2
                        po = 512 * j2
                        nc.tensor.matmul(
                            p3[:, po:po + 512], s_w3a, y2[0][:, sl:sl + 512],
                            start=True, stop=False,
                        )
                        nc.tensor.matmul(
                            p3[:, po:po + 512], s_w3b, y2[1][:, sl:sl + 512],
                            start=False, stop=True,
                        )
                    nc.vector.tensor_copy(
                        ed_t[:, 1024 * jj:1024 * jj + 1024], p3[:],
                    )
                nc.sync.dma_start(
                    out=out_d[:, st * ST:(st + 1) * ST], in_=ed_t[:],
                )

    nc.compile()
    return nc


def kernel(last_node_batch, h, W_embed, b_embed, W1, b1, W2, b2, W3, b3,
           segment_ids, max_nodes):
    global LAST_RESULT
    lnb = np.asarray(last_node_batch, np.float32)
    h = np.asarray(h, np.float32)
    seg = np.asarray(segment_ids)
    mn = int(np.asarray(max_nodes))

    expected_seg = np.repeat(np.arange(B, dtype=seg.dtype), NPG)
    if not (lnb.shape == (B, HID) and h.shape == (N, HID) and mn == NPG
            and seg.shape == (N,) and np.array_equal(seg, expected_seg)):
        return _numpy_ref(last_node_batch, h, W_embed, b_embed, W1, b1, W2, b2,
                          W3, b3, segment_ids, max_nodes)

    from concourse.bass_utils import run_bass_kernel_spmd

    if "nc" not in _CACHE:
        _CACHE["nc"] = _build()
    nc = _CACHE["nc"]

    W1 = np.asarray(W1, np.float32)
    W2 = np.asarray(W2, np.float32)
    W3 = np.asarray(W3, np.float32)
    b1v = np.asarray(b1, np.float32)
    b2v = np.asarray(b2, np.float32)
    b3v = np.asarray(b3, np.float32)
    lnbT = _round_fp32r(lnb.T)

    bpk = np.stack([
        np.asarray(b_embed, np.float32), b1v[:HID], b1v[HID:],
        b2v[:HID], b2v[HID:],
    ], axis=1)
    bpk = np.ascontiguousarray(bpk)

    wcommon = np.concatenate([
        _round_fp32r(np.asarray(W_embed, np.float32)),
        _round_fp32r(W1[:HID, :]), _round_fp32r(W1[HID:, :]),
        _round_fp32r(W2[:HID, :]), _round_fp32r(W2[HID:, :]),
        _round_fp32r(W3[:HID, :]), _round_fp32r(W3[HID:, :]),
    ], axis=1)

    in_maps = []
    for c in range(NCORES):
        wpk = np.concatenate(
            [wcommon, lnbT[:, c * GPC:(c + 1) * GPC]], axis=1)
        m = {
            "wpk": np.ascontiguousarray(wpk),
            "bpk": bpk,
            "hT": _round_fp32r(np.ascontiguousarray(h[c * NPC:(c + 1) * NPC].T)),
        }
        in_maps.append(m)

    trace = bool(int(os.environ.get("KERNEL_TRACE", "0")))
    res = run_bass_kernel_spmd(nc, in_maps, core_ids=list(range(NCORES)),
                               trace=trace)
    LAST_RESULT = res

    out = np.empty((B, NPG * 2), np.float32)
    for c in range(NCORES):
        od = res.results[c]["out"]          # [2, NPC]; [cc, n] = edges[n, cc]
        blk = od.reshape(2, GPC, NPG).transpose(1, 2, 0).reshape(GPC, NPG * 2)
        out[c * GPC:(c + 1) * GPC] = blk
    out += np.tile(b3v, NPG)[None, :]
    return out


# revision 23
# speedup vs baseline: 1.2675x; 1.2675x over previous
"""Trainium2 Bass kernel for nn_Batch_Edge (gnn_message_passing).

Computation (see reference):
    node_embed = last_node_batch @ W_embed + b_embed          # [B, H]
    stack      = concat([h, node_embed[seg]], axis=1)         # [N, 2H]
    out        = tanh(stack @ W1 + b1); out = tanh(out @ W2 + b2)
    edges      = out @ W3 + b3                                # [N, 2]
    result     = edges reshaped to [B, max_nodes*2]  (no padding: all graphs full)

Strategy: shard 512 graphs (131072 nodes) contiguously across 8 cores (64
graphs / 16384 nodes each). All activations are kept feature-on-partition
([feature, node]) so matmuls need no on-device transposes; the host supplies
h and last_node_batch pre-transposed. The per-graph embedding contribution
C = node_embed @ W1[H:, :] + b1 is computed once per core ([256, 64]) and
added to the L1 PSUM by DVE as a per-partition broadcast. Matmuls run in
fp32r (fp32 with mantissa rounded to 11 bits — full PE rate); inputs are
pre-rounded on the host so no device cycles are spent rounding.
"""

import os
import numpy as np

B = 512
NPG = 256               # nodes per graph
N = B * NPG             # 131072
HID = 128
NCORES = 8
GPC = B // NCORES       # 64 graphs per core
NPC = N // NCORES       # 16384 nodes per core
PAD_VALUE = -10000.0

ST = 2048               # supertile: nodes handled per main-loop iteration
NST = NPC // ST         # 8 supertiles per core

LAST_RESULT = None      # BassKernelResults of the most recent device run
_CACHE = {}


def _round_fp32r(x: np.ndarray) -> np.ndarray:
    """Round fp32 mantissa to 11 explicit bits (round-half-to-even) — the
    fp32r encoding the PE consumes at full rate."""
    u = np.ascontiguousarray(x, dtype=np.float32).view(np.uint32)
    lsb = (u >> np.uint32(12)) & np.uint32(1)
    r = (u + np.uint32(0x7FF) + lsb) & np.uint32(0xFFFFF000)
    return r.view(np.float32)


def _numpy_ref(last_node_batch, h, W_embed, b_embed, W1, b1, W2, b2, W3, b3,
               segment_ids, max_nodes):
    """Exact host fallback (used only if inputs don't match the expected
    uniform-graph structure)."""
    lnb = np.asarray(last_node_batch, np.float32)
    h = np.asarray(h, np.float32)
    seg = np.asarray(segment_ids).astype(np.int64)
    b = lnb.shape[0]
    n = h.shape[0]
    mn = int(np.asarray(max_nodes))
    node_embed = lnb @ np.asarray(W_embed, np.float32) + np.asarray(b_embed, np.float32)
    stack = np.concatenate([h, node_embed[seg]], axis=1)
    out = np.tanh(stack @ np.asarray(W1, np.float32) + np.asarray(b1, np.float32))
    out = np.tanh(out @ np.asarray(W2, np.float32) + np.asarray(b2, np.float32))
    edges = out @ np.asarray(W3, np.float32) + np.asarray(b3, np.float32)
    counts = np.zeros(b, np.int64)
    np.add.at(counts, seg, 1)
    offsets = np.cumsum(counts) - counts
    pos = np.arange(n) - offsets[seg]
    padded = np.full((b, mn, 2), PAD_VALUE, np.float32)
    padded[seg, pos] = edges
    return padded.reshape(b, mn * 2)


def _build():
    """Build + compile the per-core Bass program (identical on all cores)."""
    import concourse.bacc as bacc
    import concourse.mybir as mybir
    import concourse.tile as tile

    f32 = mybir.dt.float32
    f32r = mybir.dt.float32r
    Tanh = mybir.ActivationFunctionType.Tanh

    nc = bacc.Bacc("TRN2", target_bir_lowering=False, debug=False)

    # wpk layout (free dim): we[0:128] w1t[128:384] w1b[384:640] w2a[640:896]
    #                        w2b[896:1152] w3a[1152:1154] w3b[1154:1156]
    #                        lnbT[1156:1220]
    hT = nc.dram_tensor("hT", [128, NPC], f32r, kind="ExternalInput").ap()
    wpk = nc.dram_tensor("wpk", [128, 1220], f32r, kind="ExternalInput").ap()
    # bpk columns: be, b1a, b1b, b2a, b2b
    bpk = nc.dram_tensor("bpk", [128, 5], f32, kind="ExternalInput").ap()
    out_d = nc.dram_tensor("out", [2, NPC], f32, kind="ExternalOutput").ap()

    with tile.TileContext(nc) as tc:
        with (
            tc.tile_pool(name="w", bufs=1) as wp,
            tc.tile_pool(name="io", bufs=2) as io,
            tc.tile_pool(name="act", bufs=2) as ac,
            tc.tile_pool(name="ps1", bufs=2, space="PSUM") as ps1,
            tc.tile_pool(name="ps2", bufs=2, space="PSUM") as ps2,
            tc.tile_pool(name="ps3", bufs=1, space="PSUM") as ps3,
        ):
            # weights first (small, unblock the prologue), then h chunk 0
            s_w = wp.tile([128, 1220], f32r, tag="wpk")
            nc.sync.dma_start(out=s_w[:], in_=wpk)
            s_b = wp.tile([128, 5], f32, tag="bpk")
            nc.sync.dma_start(out=s_b[:], in_=bpk)
            h_tiles = {}
            t_h0 = io.tile([128, ST], f32r, tag="h")
            nc.sync.dma_start(out=t_h0[:, 0:ST // 2], in_=hT[:, 0:ST // 2])
            nc.sync.dma_start(out=t_h0[:, ST // 2:ST], in_=hT[:, ST // 2:ST])
            h_tiles[0] = t_h0
            s_we = s_w[:, 0:128]
            s_w1t = s_w[:, 128:384]
            s_w1b = s_w[:, 384:640]
            s_w2a = s_w[:, 640:896]
            s_w2b = s_w[:, 896:1152]
            s_w3a = s_w[:, 1152:1154]
            s_w3b = s_w[:, 1154:1156]
            s_lnb = s_w[:, 1156:1220]
            s_be = s_b[:, 0:1]
            s_b1 = [s_b[:, 1:2], s_b[:, 2:3]]
            s_b2 = [s_b[:, 3:4], s_b[:, 4:5]]

            # node_embed^T = W_embed.T @ lnb^T + b_embed   [128, GPC]
            p_e = ps1.tile([128, GPC], f32, tag="ps1")
            nc.tensor.matmul(p_e[:], s_we, s_lnb, start=True, stop=True)
            s_emb = wp.tile([128, GPC], f32r, tag="emb")
            nc.vector.tensor_scalar_add(s_emb[:], p_e[:], s_be)

            # C^T halves = (W1[H:, :].T @ node_embed^T + b1)  each [128, GPC]
            s_ct = []
            for m in (0, 1):
                p_c = ps1.tile([128, GPC], f32, tag="ps1")
                nc.tensor.matmul(
                    p_c[:], s_w1b[:, 128 * m:128 * m + 128], s_emb[:],
                    start=True, stop=True,
                )
                t = wp.tile([128, GPC], f32, tag=f"ct{m}")
                nc.vector.tensor_scalar_add(t[:], p_c[:], s_b1[m])
                s_ct.append(t)

            for st in range(NST):
                if st in h_tiles:
                    t_h = h_tiles.pop(st)
                else:
                    t_h = io.tile([128, ST], f32r, tag="h")
                    nc.sync.dma_start(
                        out=t_h[:], in_=hT[:, st * ST:(st + 1) * ST],
                    )

                # L1: y1[m] = tanh(W1[:H, m].T @ h^T + C[m][:, g]);
                # both halves staged into one tile so tanh is a single
                # [128, 4096] ACT instruction.
                y1s = ac.tile([128, 2 * ST], f32r, tag="y1s")
                for m in (0, 1):
                    for j in range(ST // 512):
                        p1 = ps1.tile([128, 512], f32, tag="ps1")
                        nc.tensor.matmul(
                            p1[:], s_w1t[:, 128 * m:128 * m + 128],
                            t_h[:, 512 * j:512 * j + 512],
                            start=True, stop=True,
                        )
                        g = st * (ST // NPG) + j * 2
                        o = m * ST + 512 * j
                        nc.vector.tensor_tensor(
                            y1s[:, o:o + 512]
                            .rearrange("p (a b) -> p a b", a=2),
                            p1[:].rearrange("p (a b) -> p a b", a=2),
                            s_ct[m][:, g:g + 2].broadcast_to((128, 2, 256)),
                            mybir.AluOpType.add,
                        )
                y1t = ac.tile([128, 2 * ST], f32r, tag="y1")
                for ci in (0, 1):
                    nc.scalar.activation(
                        y1t[:].rearrange("p (a b) -> p a b", a=4)[:, ci::2, :],
                        y1s[:].rearrange("p (a b) -> p a b", a=4)[:, ci::2, :],
                        Tanh,
                    )
                y1 = [y1t[:, 0:ST], y1t[:, ST:2 * ST]]

                # L2: y2[m] = tanh(W2[:, m].T @ y1 + b2[m])
                y2 = []
                for m in (0, 1):
                    yt = ac.tile([128, ST], f32r, tag=f"y2{m}")
                    for jj in range(ST // 1024):
                        p2 = ps2.tile([128, 1024], f32, tag="ps2")
                        for j2 in (0, 1):
                            sl = 1024 * jj + 512 * j2
                            po = 512 * j2
                            nc.tensor.matmul(
                                p2[:, po:po + 512],
                                s_w2a[:, 128 * m:128 * m + 128],
                                y1[0][:, sl:sl + 512],
                                start=True, stop=False,
                            )
                            nc.tensor.matmul(
                                p2[:, po:po + 512],
                                s_w2b[:, 128 * m:128 * m + 128],
                                y1[1][:, sl:sl + 512],
                                start=False, stop=True,
                            )
                        nc.scalar.activation(
                            yt[:, 1024 * jj:1024 * jj + 1024], p2[:],
                            Tanh, bias=s_b2[m],
                        )
                    y2.append(yt)

                # L3: edges^T[:, n] = W3.T @ y2[:, n]; two 512-col pairs per
                # [2, 1024] PSUM tile, one DVE copy each.
                ed_t = io.tile([2, ST], f32, tag="out")
                for jj in range(ST // 1024):
                    p3 = ps3.tile([2, 1024], f32, tag="ps3")
                    for j2 in (0, 1):
                        sl = 1024 * jj + 512 * j2
                        po = 512 * j2
                        nc.tensor.matmul(
                            p3[:, po:po + 512], s_w3a, y2[0][:, sl:sl + 512],
                            start=True, stop=False,
                        )
                        nc.tensor.matmul(
                            p3[:, po:po + 512], s_w3b, y2[1][:, sl:sl + 512],
                            start=False, stop=True,
                        )
                    nc.vector.tensor_copy(
                        ed_t[:, 1024 * jj:1024 * jj + 1024], p3[:],
                    )
                nc.sync.dma_start(
                    out=out_d[:, st * ST:(st + 1) * ST], in_=ed_t[:],
                )

    nc.compile()
    return nc


def kernel(last_node_batch, h, W_embed, b_embed, W1, b1, W2, b2, W3, b3,
           segment_ids, max_nodes):
    global LAST_RESULT
    lnb = np.asarray(last_node_batch, np.float32)
    h = np.asarray(h, np.float32)
    seg = np.asarray(segment_ids)
    mn = int(np.asarray(max_nodes))

    expected_seg = np.repeat(np.arange(B, dtype=seg.dtype), NPG)
    if not (lnb.shape == (B, HID) and h.shape == (N, HID) and mn == NPG
            and seg.shape == (N,) and np.array_equal(seg, expected_seg)):
        return _numpy_ref(last_node_batch, h, W_embed, b_embed, W1, b1, W2, b2,
                          W3, b3, segment_ids, max_nodes)

    from concourse.bass_utils import run_bass_kernel_spmd

    if "nc" not in _CACHE:
        _CACHE["nc"] = _build()
    nc = _CACHE["nc"]

    W1 = np.asarray(W1, np.float32)
    W2 = np.asarray(W2, np.float32)
    W3 = np.asarray(W3, np.float32)
    b1v = np.asarray(b1, np.float32)
    b2v = np.asarray(b2, np.float32)
    b3v = np.asarray(b3, np.float32)
    lnbT = _round_fp32r(lnb.T)

    bpk = np.stack([
        np.asarray(b_embed, np.float32), b1v[:HID], b1v[HID:],
        b2v[:HID], b2v[HID:],
    ], axis=1)
    bpk = np.ascontiguousarray(bpk)

    wcommon = np.concatenate([
        _round_fp32r(np.asarray(W_embed, np.float32)),
        _round_fp32r(W1[:HID, :]), _round_fp32r(W1[HID:, :]),
        _round_fp32r(W2[:HID, :]), _round_fp32r(W2[HID:, :]),
        _round_fp32r(W3[:HID, :]), _round_fp32r(W3[HID:, :]),
    ], axis=1)

    in_maps = []
    for c in range(NCORES):
        wpk = np.concatenate(
            [wcommon, lnbT[:, c * GPC:(c + 1) * GPC]], axis=1)
        m = {
            "wpk": np.ascontiguousarray(wpk),
            "bpk": bpk,
            "hT": _round_fp32r(np.ascontiguousarray(h[c * NPC:(c + 1) * NPC].T)),
        }
        in_maps.append(m)

    trace = bool(int(os.environ.get("KERNEL_TRACE", "0")))
    res = run_bass_kernel_spmd(nc, in_maps, core_ids=list(range(NCORES)),
                               trace=trace)
    LAST_RESULT = res

    out = np.empty((B, NPG * 2), np.float32)
    for c in range(NCORES):
        od = res.results[c]["out"]          # [2, NPC]; [cc, n] = edges[n, cc]
        blk = od.reshape(2, GPC, NPG).transpose(1, 2, 0).reshape(GPC, NPG * 2)
        out[c * GPC:(c + 1) * GPC] = blk
    out += np.tile(b3v, NPG)[None, :]
    return out
